# revision 12
# baseline (speedup 1.0000x reference)
"""Trainium2 Bass kernel for nn_DeepEC_KAN (DeepEC conv->maxpool->BN->LN->KAN x2).

Data parallel over batch (256 -> 32 per core on 8 cores). Per core:
  - host builds the 6-tap im2col patch [126, 32, 1008] (f32r); loaded in
    4-sample chunks via SWDGE (16-engine split) DMAs.
  - conv1d(K=4/8/16) = sum over tap-groups of f32r matmuls at column
    offsets 0/6/12 into the shared patch; maxpool via fused
    tensor_tensor_reduce (ACT pre-copies the tail window to SBUF, DVE
    max-pairs + reduces in one pass); BN1..4 + conv bias folded into
    per-channel affine on host.
  - LayerNorm stats via ones-vector matmuls (cross-partition sums on PE).
  - KAN: silu on ACT; cubic B-spline bases via closed-form cardinal spline
    (interval indicators + 4 local cubics), DVE polys + GPSIMD assembly;
    contraction matmuls (f32r) with samples as the stationary operand.
  - tail (LN+KAN) runs in two half-batches so it overlaps the conv phase.
"""

import sys
import numpy as np

sys.path.insert(0, "/opt/trn_rl_repo")

import concourse.bass as bass  # noqa: E402
import concourse.bacc as bacc  # noqa: E402
import concourse.tile as tile  # noqa: E402
from concourse import mybir  # noqa: E402
from concourse.bass_utils import run_bass_kernel_spmd  # noqa: E402

F32 = mybir.dt.float32
F32R = mybir.dt.float32r
ALU = mybir.AluOpType
ACTF = mybir.ActivationFunctionType
AX = mybir.AxisListType

NCORES = 8
B = 256
BC = B // NCORES  # 32 samples per core
C = 21
L = 1000
LP = 1008
CH = 4                 # samples per patch chunk
NCHUNK = BC // CH      # 8 chunks
NH = BC // 2           # tail half-batch (16)
CONV_L = [997, 993, 985]
GROUPS = [
    (0, 84, 0, 0, True, True),      # conv1 taps 0-3
    (84, 126, 0, 1, True, False),   # conv2 taps 0-5
    (210, 42, 6, 1, False, True),   # conv2 taps 6-7
    (252, 126, 0, 2, True, False),  # conv3 taps 0-5
    (378, 126, 6, 2, False, False),  # conv3 taps 6-11
    (504, 84, 12, 2, False, True),  # conv3 taps 12-15
]
WCONV_ROWS = 588
NW1 = 21
NW2 = 28
D1OUT = 512
D2OUT = 229
W2PAD = 256
USE_TTR = False


def _emit_splines(nc, pool, x2d, W, tag):
    """bs [128, 6, W] (f32r) <- cubic B-spline bases of x2d [128, W]."""
    t = pool.tile([128, W], F32, tag=f"{tag}_t", name=f"{tag}_t")
    nc.vector.tensor_scalar(out=t, in0=x2d, scalar1=1.5, scalar2=4.5,
                            op0=ALU.mult, op1=ALU.add)
    st = pool.tile([128, 10, W], F32, tag=f"{tag}_st", name=f"{tag}_st")
    for k in range(10):
        nc.vector.tensor_scalar(out=st[:, k], in0=t, scalar1=float(k),
                                scalar2=None, op0=ALU.is_ge)
    kf = pool.tile([128, W], F32, tag=f"{tag}_kf", name=f"{tag}_kf")
    stv = st.rearrange("p k w -> p w k")[:, :, 1:9]
    nc.vector.reduce_sum(out=kf, in_=stv, axis=AX.X)
    u = pool.tile([128, W], F32, tag=f"{tag}_u", name=f"{tag}_u")
    nc.vector.tensor_sub(u, t, kf)
    u2 = pool.tile([128, W], F32, tag=f"{tag}_u2", name=f"{tag}_u2")
    nc.vector.tensor_mul(u2, u, u)
    u3 = pool.tile([128, W], F32, tag=f"{tag}_u3", name=f"{tag}_u3")
    nc.vector.tensor_mul(u3, u2, u)
    w1 = pool.tile([128, W], F32, tag=f"{tag}_w1", name=f"{tag}_w1")
    nc.vector.tensor_scalar(out=w1, in0=u, scalar1=-1.0, scalar2=1.0,
                            op0=ALU.mult, op1=ALU.add)
    w2 = pool.tile([128, W], F32, tag=f"{tag}_w2", name=f"{tag}_w2")
    nc.vector.tensor_mul(w2, w1, w1)
    D3 = pool.tile([128, W], F32, tag=f"{tag}_D3", name=f"{tag}_D3")
    nc.vector.tensor_mul(D3, w2, w1)
    D0 = u3
    u26 = pool.tile([128, W], F32, tag=f"{tag}_u26", name=f"{tag}_u26")
    nc.vector.tensor_scalar_mul(u26, u2, 6.0)
    D2 = pool.tile([128, W], F32, tag=f"{tag}_D2", name=f"{tag}_D2")
    nc.vector.scalar_tensor_tensor(out=D2, in0=u3, scalar=3.0, in1=u26,
                                   op0=ALU.mult, op1=ALU.subtract)
    nc.vector.tensor_scalar_add(D2, D2, 4.0)
    D1 = pool.tile([128, W], F32, tag=f"{tag}_D1", name=f"{tag}_D1")
    nc.vector.tensor_add(D1, D0, D2)
    nc.vector.tensor_add(D1, D1, D3)
    nc.vector.tensor_scalar(out=D1, in0=D1, scalar1=-1.0, scalar2=6.0,
                            op0=ALU.mult, op1=ALU.add)
    q = pool.tile([128, 9, W], F32, tag=f"{tag}_q", name=f"{tag}_q")
    nc.vector.tensor_sub(q, st[:, 0:9], st[:, 1:10])
    Ds = [D0, D1, D2, D3]
    bs = pool.tile([128, 6, W], F32R, tag=f"{tag}_bs", name=f"{tag}_bs")
    tmpA = pool.tile([128, W], F32, tag=f"{tag}_tmpA", name=f"{tag}_tmpA")
    tmpB = pool.tile([128, W], F32, tag=f"{tag}_tmpB", name=f"{tag}_tmpB")
    for i in range(6):
        eng = nc.vector if i < 3 else nc.gpsimd
        tmp = tmpA if i < 3 else tmpB
        eng.tensor_mul(bs[:, i], q[:, i], Ds[0])
        for j in range(1, 4):
            eng.tensor_mul(tmp, q[:, i + j], Ds[j])
            eng.tensor_add(bs[:, i], bs[:, i], tmp)
    return bs


def _build_program():
    nc = bacc.Bacc("TRN2", target_bir_lowering=False, debug=False,
                   num_devices=NCORES)
    patch_d = nc.dram_tensor("patch", [126, BC, LP], F32R,
                             kind="ExternalInput").ap()
    wconv = nc.dram_tensor("wconv", [WCONV_ROWS, 128], F32R,
                           kind="ExternalInput").ap()
    kconst = nc.dram_tensor("kconst", [128, 5, 96], F32,
                            kind="ExternalInput").ap()
    w1s_d = nc.dram_tensor("w1s", [128, NW1, D1OUT], F32R,
                           kind="ExternalInput").ap()
    w2s_d = nc.dram_tensor("w2s", [128, NW2, W2PAD], F32R,
                           kind="ExternalInput").ap()
    id32_d = nc.dram_tensor("id32", [32, 32], F32, kind="ExternalInput").ap()
    out_d = nc.dram_tensor("out", [BC, D2OUT], F32, kind="ExternalOutput").ap()

    with tile.TileContext(nc) as tc:
        with (
            tc.tile_pool(name="const", bufs=1) as const,
            tc.tile_pool(name="patches", bufs=2) as patches,
            tc.tile_pool(name="work", bufs=1) as work,
            tc.tile_pool(name="redu", bufs=4) as redu,
            tc.tile_pool(name="psconv", bufs=1, space="PSUM") as psconv,
            tc.tile_pool(name="pstail", bufs=1, space="PSUM") as pstail,
        ):
            # ---- constants ----
            wc_tiles = []
            for gi, (r0, nr, _off, _cj, _f, _l) in enumerate(GROUPS):
                wt = const.tile([128, 128], F32R, tag=f"wc{gi}", name=f"wc{gi}")
                nc.sync.dma_start(out=wt[0:nr, :], in_=wconv[r0:r0 + nr, :])
                wc_tiles.append(wt)
            kc = const.tile([128, 5, 96], F32, tag="kc", name="kc")
            nc.sync.dma_start(out=kc, in_=kconst)
            w1s = const.tile([128, NW1, D1OUT], F32R, tag="w1s", name="w1s")
            nc.gpsimd.dma_start(out=w1s, in_=w1s_d)
            w2s = const.tile([128, NW2, W2PAD], F32R, tag="w2s", name="w2s")
            nc.gpsimd.dma_start(out=w2s, in_=w2s_d)
            id32 = const.tile([32, 32], F32, tag="id32", name="id32")
            nc.sync.dma_start(out=id32, in_=id32_d)
            ones = const.tile([128, 128], F32, tag="ones", name="ones")
            nc.vector.memset(ones, 1.0)

            mraw = work.tile([128, 3, BC], F32, tag="mraw", name="mraw")
            kc3 = kc.rearrange("p i (j b) -> p i j b", j=3)

            def emit_tail(b0, hx):
                nb = NH
                W1W = 3 * nb
                sfx = f"h{hx}"
                mrh = mraw[:, :, b0:b0 + nb]
                kch = kc3[:, :, :, b0:b0 + nb]
                t96 = work.tile([128, 3, nb], F32, tag=f"t96{sfx}", name=f"t96{sfx}")
                nc.gpsimd.tensor_add(t96, mrh, kch[:, 0])
                h96 = work.tile([128, 3, nb], F32, tag=f"h96{sfx}", name=f"h96{sfx}")
                nc.gpsimd.tensor_scalar_max(h96, t96, 0.0)
                nc.gpsimd.tensor_mul(h96, h96, kch[:, 1])
                nc.gpsimd.tensor_add(h96, h96, kch[:, 2])
                sq96 = work.tile([128, 3, nb], F32, tag=f"sq96{sfx}", name=f"sq96{sfx}")
                nc.scalar.activation(out=sq96, in_=h96, func=ACTF.Square)
                psLN = pstail.tile([1, 4 * W1W], F32, tag="small", name=f"psLN{sfx}")
                nc.tensor.matmul(out=psLN[0:1, 0:W1W], lhsT=ones[:, 0:1],
                                 rhs=h96, start=True, stop=True)
                nc.tensor.matmul(out=psLN[0:1, W1W:2 * W1W], lhsT=ones[:, 0:1],
                                 rhs=sq96, start=True, stop=True)
                sums = work.tile([1, 2, nb], F32, tag=f"sums{sfx}", name=f"sums{sfx}")
                psLNv = psLN[0:1, 0:2 * W1W].rearrange("p (x j b) -> p x b j", x=2, j=3)
                nc.vector.reduce_sum(out=sums[0:1, 0], in_=psLNv[0:1, 0], axis=AX.X)
                nc.vector.reduce_sum(out=sums[0:1, 1], in_=psLNv[0:1, 1], axis=AX.X)
                muinv = work.tile([1, 2, nb], F32, tag=f"muinv{sfx}", name=f"muinv{sfx}")
                nc.vector.tensor_scalar_mul(muinv[0:1, 0], sums[0:1, 0], 1.0 / 384)
                msq = work.tile([1, nb], F32, tag=f"msq{sfx}", name=f"msq{sfx}")
                nc.vector.tensor_mul(msq, muinv[0:1, 0], muinv[0:1, 0])
                var = work.tile([1, nb], F32, tag=f"var{sfx}", name=f"var{sfx}")
                nc.vector.scalar_tensor_tensor(out=var, in0=sums[0:1, 1],
                                               scalar=1.0 / 384, in1=msq,
                                               op0=ALU.mult, op1=ALU.subtract)
                nc.vector.tensor_scalar_add(var, var, 1e-5)
                sd = work.tile([1, nb], F32, tag=f"sd{sfx}", name=f"sd{sfx}")
                nc.scalar.activation(out=sd, in_=var, func=ACTF.Sqrt, bias=0.0)
                nc.vector.reciprocal(muinv[0:1, 1], sd)
                psB = pstail.tile([128, 2, nb], F32, tag="small", name=f"psB{sfx}")
                nc.tensor.matmul(out=psB, lhsT=ones[0:1, :], rhs=muinv[0:1],
                                 start=True, stop=True)
                muinvB = work.tile([128, 2, nb], F32, tag=f"muinvB{sfx}",
                                   name=f"muinvB{sfx}")
                nc.scalar.copy(out=muinvB, in_=psB)
                hn = work.tile([128, 3, nb], F32, tag=f"hn{sfx}", name=f"hn{sfx}")
                for j in range(3):
                    nc.gpsimd.tensor_sub(hn[:, j], h96[:, j], muinvB[:, 0])
                    nc.gpsimd.tensor_mul(hn[:, j], hn[:, j], muinvB[:, 1])
                nc.gpsimd.tensor_mul(hn, hn, kch[:, 3])
                nc.gpsimd.tensor_add(hn, hn, kch[:, 4])

                # KAN layer 1
                hn2d = hn.rearrange("p j b -> p (j b)")
                sil = work.tile([128, W1W], F32R, tag=f"sil{sfx}", name=f"sil{sfx}")
                nc.scalar.activation(out=sil, in_=hn2d, func=ACTF.Sigmoid)
                nc.vector.tensor_mul(sil, sil, hn2d)
                bs1 = _emit_splines(nc, work, hn2d, W1W, f"sp1{sfx}")
                psK1 = pstail.tile([nb, D1OUT], F32, tag="big", name=f"psK1{sfx}")
                mi = 0
                for j in range(3):
                    nc.tensor.matmul(out=psK1, lhsT=sil[:, j * nb:(j + 1) * nb],
                                     rhs=w1s[:, j], start=(mi == 0),
                                     stop=(mi == NW1 - 1))
                    mi += 1
                for j in range(3):
                    for g in range(6):
                        nc.tensor.matmul(out=psK1,
                                         lhsT=bs1[:, g, j * nb:(j + 1) * nb],
                                         rhs=w1s[:, 3 + j * 6 + g],
                                         start=(mi == 0), stop=(mi == NW1 - 1))
                        mi += 1
                h2s = work.tile([nb, D1OUT], F32, tag=f"h2s{sfx}", name=f"h2s{sfx}")
                nc.scalar.copy(out=h2s, in_=psK1)

                psT = pstail.tile([128, 4 * nb], F32, tag="big", name=f"psT{sfx}")
                for j in range(4):
                    nc.tensor.transpose(out=psT[:, j * nb:(j + 1) * nb],
                                        in_=h2s[:, j * 128:(j + 1) * 128],
                                        identity=id32[0:nb, 0:nb])
                h2T = work.tile([128, 4 * nb], F32, tag=f"h2T{sfx}", name=f"h2T{sfx}")
                nc.scalar.copy(out=h2T, in_=psT)

                # KAN layer 2
                W2W = 4 * nb
                sil2 = work.tile([128, W2W], F32R, tag=f"sil2{sfx}", name=f"sil2{sfx}")
                nc.scalar.activation(out=sil2, in_=h2T, func=ACTF.Sigmoid)
                nc.vector.tensor_mul(sil2, sil2, h2T)
                bs2 = _emit_splines(nc, work, h2T, W2W, f"sp2{sfx}")
                psK2 = pstail.tile([nb, W2PAD], F32, tag="big", name=f"psK2{sfx}")
                mi = 0
                for j in range(4):
                    nc.tensor.matmul(out=psK2, lhsT=sil2[:, j * nb:(j + 1) * nb],
                                     rhs=w2s[:, j], start=(mi == 0),
                                     stop=(mi == NW2 - 1))
                    mi += 1
                for j in range(4):
                    for g in range(6):
                        nc.tensor.matmul(out=psK2,
                                         lhsT=bs2[:, g, j * nb:(j + 1) * nb],
                                         rhs=w2s[:, 4 + j * 6 + g],
                                         start=(mi == 0), stop=(mi == NW2 - 1))
                        mi += 1
                outS = work.tile([nb, D2OUT], F32, tag=f"outS{sfx}", name=f"outS{sfx}")
                nc.scalar.copy(out=outS, in_=psK2[:, 0:D2OUT])
                nc.sync.dma_start(out=out_d[b0:b0 + nb], in_=outS)

            # ---- conv phase: chunks of CH samples, tail per half ----
            for c in range(NCHUNK):
                ptile = patches.tile([128, CH, LP], F32R, name="ptile")
                nc.gpsimd.dma_start(out=ptile[0:126],
                                    in_=patch_d[:, c * CH:(c + 1) * CH, :])
                for bi in range(CH):
                    b = c * CH + bi
                    pc = [psconv.tile([128, 1024], F32, tag=f"pc{j}",
                                      name=f"pc{j}") for j in range(3)]
                    for gi, (r0, nr, off, cj, first, last) in enumerate(GROUPS):
                        lcj = CONV_L[cj] + (CONV_L[cj] & 1)
                        for (n0, n1) in ((0, 512), (512, lcj)):
                            nc.tensor.matmul(
                                out=pc[cj][:, n0:n1],
                                lhsT=wc_tiles[gi][0:nr, :],
                                rhs=ptile[0:nr, bi, off + n0: off + n1],
                                start=first, stop=last,
                            )
                    for cj in range(3):
                        lcj = CONV_L[cj]
                        if USE_TTR:
                            scr = redu.tile([128, 512], F32, tag="scr", name="scr")
                            nc.scalar.copy(out=scr, in_=pc[cj][:, lcj - 512:lcj])
                            dmy = redu.tile([128, 512], F32, tag="dmy", name="dmy")
                            nc.vector.tensor_tensor_reduce(
                                out=dmy, in0=pc[cj][:, 0:512], in1=scr,
                                scale=1.0, scalar=-1e30, op0=ALU.max, op1=ALU.max,
                                accum_out=mraw[:, cj, b:b + 1])
                        else:
                            nc.vector.reduce_max(out=mraw[:, cj, b:b + 1],
                                                 in_=pc[cj][:, 0:lcj],
                                                 axis=AX.X)
                if c == NCHUNK // 2 - 1:
                    emit_tail(0, 0)
            emit_tail(NH, 1)
    nc.compile()
    return nc


def _host_prep(inputs):
    f = np.float32
    x = np.asarray(inputs["x"], f)
    xT = np.ascontiguousarray(x.transpose(0, 2, 1))  # [B, 21, 1000]
    xTpad = np.zeros((B, C, LP + 8), f)
    xTpad[:, :, :L] = xT
    patches = []
    for i in range(NCORES):
        sh = xTpad[i * BC:(i + 1) * BC]
        p = np.empty((6, C, BC, LP), f)
        for s in range(6):
            p[s] = sh[:, :, s:s + LP].transpose(1, 0, 2)
        patches.append(np.ascontiguousarray(p.reshape(126, BC, LP)))

    def chunks(w, taps):
        return [np.ascontiguousarray(
            np.asarray(w, f)[:, :, t0:t1].transpose(2, 1, 0).reshape((t1 - t0) * C, 128))
            for t0, t1 in taps]

    wconv = np.concatenate(
        chunks(inputs["conv1_w"], [(0, 4)])
        + chunks(inputs["conv2_w"], [(0, 6), (6, 8)])
        + chunks(inputs["conv3_w"], [(0, 6), (6, 12), (12, 16)]), 0)

    def fold(p):
        g, bb, m, v = (np.asarray(inputs[p + s], f) for s in ("_g", "_b", "_m", "_v"))
        s = g / np.sqrt(v + 1e-5)
        return s, bb - m * s

    s1, t1 = fold("bn1")
    s2, t2 = fold("bn2")
    s3, t3 = fold("bn3")
    s4, t4 = fold("bn4")
    Sall = np.concatenate([s1, s2, s3]) * s4
    Tall = np.concatenate([t1, t2, t3]) * s4 + t4
    cb = np.concatenate([np.asarray(inputs["conv1_b"], f),
                         np.asarray(inputs["conv2_b"], f),
                         np.asarray(inputs["conv3_b"], f)])

    def expand(v):
        return np.repeat(np.asarray(v, f).reshape(3, 128).T[:, :, None], BC, 2)

    kconst = np.stack([expand(cb), expand(Sall), expand(Tall),
                       expand(np.asarray(inputs["ln_g"], f)),
                       expand(np.asarray(inputs["ln_b"], f))], 1)
    kconst = np.ascontiguousarray(kconst.reshape(128, 5, 96))

    bw1 = np.asarray(inputs["base_w1"], f)
    sw1 = np.asarray(inputs["spline_w1"], f) / 6.0
    w1s = np.empty((128, NW1, D1OUT), f)
    for j in range(3):
        w1s[:, j, :] = bw1[:, j * 128:(j + 1) * 128].T
        for g in range(6):
            w1s[:, 3 + j * 6 + g, :] = sw1[:, j * 128:(j + 1) * 128, g].T
    bw2 = np.asarray(inputs["base_w2"], f)
    sw2 = np.asarray(inputs["spline_w2"], f) / 6.0
    w2s = np.zeros((128, NW2, W2PAD), f)
    for j in range(4):
        w2s[:, j, :D2OUT] = bw2[:, j * 128:(j + 1) * 128].T
        for g in range(6):
            w2s[:, 4 + j * 6 + g, :D2OUT] = sw2[:, j * 128:(j + 1) * 128, g].T

    shared = {
        "wconv": np.ascontiguousarray(wconv),
        "kconst": kconst,
        "w1s": np.ascontiguousarray(w1s),
        "w2s": np.ascontiguousarray(w2s),
        "id32": np.eye(32, dtype=f),
    }
    return shared, patches


_NC_CACHE = None


def _get_nc():
    global _NC_CACHE
    if _NC_CACHE is None:
        _NC_CACHE = _build_program()
    return _NC_CACHE


def make_in_maps(inputs):
    shared, patches = _host_prep(inputs)
    return [{**shared, "patch": patches[i]} for i in range(NCORES)]


def kernel(**inputs):
    nc = _get_nc()
    in_maps = make_in_maps(inputs)
    res = run_bass_kernel_spmd(nc, in_maps, list(range(NCORES)))
    return np.concatenate([res.results[i]["out"] for i in range(NCORES)], 0)
